# revision 36
# baseline (speedup 1.0000x reference)
"""BERT-base (12-layer, B=8, S=512, D=768, H=12, F=3072) forward pass on 8
Trainium2 NeuronCores.

Strategy: data-parallel over batch (1 sequence per core, no collectives).
Per core, activations are kept feature-major xT[D, S] in SBUF so that:
  - all big matmuls run as lhsT[dK,128] @ rhs[dK,512] at full PE rate
  - LayerNorm statistics (sums over the feature dim = partition dim) are
    ones-vector matmuls on the PE
  - softmax runs in scoresT [k, q] layout: the attention mask is a
    per-partition exp() bias

Pipeline structure (v8):
  - PV packed 2 heads per PSUM bank via col-group tile_position (0,0)/(0,64)
  - softmax denominators via ones[128,64] matmuls -> 64-row-replicated
    [128,S] sums per pair; Ln -> Exp(-1) on ACT gives the reciprocal tile;
    ONE tensor_tensor multiply normalizes the whole head pair
  - V bias fused into the PSUM->SBUF drain against a host-prebroadcast tile
  - Wo accumulation split into cT-readiness waves so the PE stays dense
    through the pair-4/5 reciprocal chains; kt-granular emission interleave
    keeps score matmuls ahead of the ACT exp pipeline
  - LN: variance via st1 - st0^2/D (eps dropped, 0.5*ln(D) folded into the
    Exp bias); gamma applied via scalar_tensor_tensor; LN2/emb-LN beta
    folded host-side into the next layer's QKV biases and residual bias;
    mean-subtract adds run on the otherwise-idle GPSIMD engine off an
    SBUF copy of the -mu broadcast; LN2 squares computed straight off the
    PSUM y-tiles with +b2 as the ACT Square bias
  - embedding: type-embedding gather replaced by a rank-1 STT update
    (temb[0] folded into pos_emb host-side); input DMAs on the scalar
    queue; output DMA'd as bf16 and upcast host-side
  - deterministic 8-bank PSUM tag plan: a0,a1,s0,s1,c0,c1,d0,d1
"""
import numpy as np

import concourse.bass as bass
import concourse.mybir as mybir
import concourse.tile as tile
from concourse import bass_utils
from concourse.masks import make_identity

AF = mybir.ActivationFunctionType
OP = mybir.AluOpType
F32 = mybir.dt.float32
F32R = mybir.dt.float32r
BF16 = mybir.dt.bfloat16
I32 = mybir.dt.int32

B, S, D, H, F, L, V = 8, 512, 768, 12, 3072, 12, 30522
DK = D // H
SCALE = 1.0 / float(np.sqrt(DK))
NT = D // 128      # 6 feature tiles
NTF = F // 128     # 24 ffn tiles
NST = S // 128     # 4 sequence tiles
NP = H // 2        # 6 head pairs

_NC_CACHE = None


# ---------------------------------------------------------------------------
# wait-slot legalization: walrus codegen allows only ONE sync-wait command on
# TPB instructions; hoist excess waits into standalone EventSemaphores.
def _legalize_waits(nc):
    skip = (mybir.InstEventSemaphore, mybir.InstNoOp)
    n = 0
    for fn in nc.m.functions:
        for blk in fn.blocks:
            out = []
            for inst in blk.instructions:
                si = inst.sync_info
                if si is not None and si.on_wait and not isinstance(inst, skip) \
                        and len(si.on_wait) > 1:
                    waits = list(si.on_wait)
                    for j, w in enumerate(waits[:-1]):
                        ev = mybir.InstEventSemaphore(
                            name=f"{inst.name}-lgw{j}", ins=[], outs=[],
                            sync_info=mybir.SyncInfo(on_wait=[w], on_update=[]),
                        )
                        ev.engine = inst.engine
                        out.append(ev)
                        n += 1
                    inst.sync_info = mybir.SyncInfo(
                        on_wait=[waits[-1]], on_update=list(si.on_update))
                out.append(inst)
            try:
                blk.instructions = out
            except Exception:
                blk.instructions.clear()
                blk.instructions.extend(out)
    return n


def _build_nc():
    nc = bass.Bass("TRN2", target_bir_lowering=False, debug=False,
                   enable_asserts=False, num_devices=8)

    # ---- DRAM I/O ---------------------------------------------------------
    d_ids = nc.dram_tensor("ids", [S, 1], I32, kind="ExternalInput")
    d_tti = nc.dram_tensor("tti", [S, 1], F32, kind="ExternalInput")
    d_mask = nc.dram_tensor("maskadd", [S], F32, kind="ExternalInput")
    d_wemb = nc.dram_tensor("wemb", [V, D], F32, kind="ExternalInput")
    d_pemb = nc.dram_tensor("pemb", [S, D], F32, kind="ExternalInput")
    d_dtb = nc.dram_tensor("dtb", [128, D], BF16, kind="ExternalInput")
    d_eg = nc.dram_tensor("eg", [D], F32, kind="ExternalInput")
    # pre-tiled weights: each tile is one contiguous-per-partition DMA
    d_wq = nc.dram_tensor("wq", [L, NT, 128, NT, 128], BF16, kind="ExternalInput")
    d_wk = nc.dram_tensor("wk", [L, NT, 128, NT, 128], BF16, kind="ExternalInput")
    d_wv = nc.dram_tensor("wv", [L, 3, 128, NT, 256], BF16, kind="ExternalInput")
    d_wo = nc.dram_tensor("wo", [L, NT, 128, NT, 128], BF16, kind="ExternalInput")
    d_w1 = nc.dram_tensor("w1", [L, NTF // 2, 128, NT, 256], BF16, kind="ExternalInput")
    d_w2 = nc.dram_tensor("w2", [L, NTF // 2, 128, 2, D], BF16, kind="ExternalInput")
    d_bq = nc.dram_tensor("bq", [L, D], F32, kind="ExternalInput")
    d_bk = nc.dram_tensor("bk", [L, D], F32, kind="ExternalInput")
    d_bvb = nc.dram_tensor("bvb", [L, 128, D], BF16, kind="ExternalInput")
    d_bo = nc.dram_tensor("bo", [L, D], F32, kind="ExternalInput")
    d_b2 = nc.dram_tensor("b2", [L, D], F32, kind="ExternalInput")  # b2 + W2@b1'
    d_fg = nc.dram_tensor("fg", [L, D], F32, kind="ExternalInput")
    d_fb = nc.dram_tensor("fb", [L, D], F32, kind="ExternalInput")  # last layer only
    d_ones = nc.dram_tensor("ones128", [128], F32, kind="ExternalInput")
    d_onesb = nc.dram_tensor("ones128b", [128], BF16, kind="ExternalInput")
    d_ones64 = nc.dram_tensor("ones64", [128, 64], BF16, kind="ExternalInput")
    d_neg1 = nc.dram_tensor("neg1", [1, 128], F32, kind="ExternalInput")
    d_out = nc.dram_tensor("out", [S, D], BF16, kind="ExternalOutput")

    with tile.TileContext(nc) as tc:
        _emit(nc, tc, locals())
    _legalize_waits(nc)
    return nc


def _emit(nc, tc, d):
    import contextlib
    ctx = contextlib.ExitStack()
    with ctx:
        _emit_body(nc, tc, d, ctx)


def _emit_body(nc, tc, d, ctx):
    pool = ctx.enter_context(tc.tile_pool(name="persist", bufs=1))
    wqkpool = ctx.enter_context(tc.tile_pool(name="wqk", bufs=6))
    wvpool = ctx.enter_context(tc.tile_pool(name="wv", bufs=3))
    wopool = ctx.enter_context(tc.tile_pool(name="wo", bufs=6))
    w1pool = ctx.enter_context(tc.tile_pool(name="w1", bufs=4))
    w2pool = ctx.enter_context(tc.tile_pool(name="w2", bufs=4))
    ppool = ctx.enter_context(tc.tile_pool(name="params", bufs=2))
    epool = ctx.enter_context(tc.tile_pool(name="epool", bufs=20))
    hpool = ctx.enter_context(tc.tile_pool(name="hpool", bufs=6))
    spool = ctx.enter_context(tc.tile_pool(name="smalls", bufs=1))
    # 8 PSUM banks, one pool per tag pair:
    ps_a = ctx.enter_context(tc.tile_pool(name="ps_a", bufs=1, space="PSUM"))
    ps_s = ctx.enter_context(tc.tile_pool(name="ps_s", bufs=1, space="PSUM"))
    ps_c = ctx.enter_context(tc.tile_pool(name="ps_c", bufs=1, space="PSUM"))
    ps_d = ctx.enter_context(tc.tile_pool(name="ps_d", bufs=1, space="PSUM"))

    # ---- persistent constants --------------------------------------------
    ones_col = pool.tile([128, 1], F32R, name="ones_col")
    nc.sync.dma_start(ones_col[:], d["d_ones"].ap().rearrange("(p o) -> p o", o=1).bitcast(F32R))
    ones_colb = pool.tile([128, 1], BF16, name="ones_colb")
    nc.sync.dma_start(ones_colb[:], d["d_onesb"].ap().rearrange("(p o) -> p o", o=1))
    one_row = pool.tile([1, 128], F32R, name="one_row")
    nc.sync.dma_start(one_row[:], d["d_ones"].ap().rearrange("(o p) -> o p", o=1).bitcast(F32R))
    neg_row = pool.tile([1, 128], F32R, name="neg_row")
    nc.sync.dma_start(neg_row[:], d["d_neg1"].ap()[:, :].bitcast(F32R))
    halfln = pool.tile([1, 1], F32, name="halfln")
    nc.vector.memset(halfln[:], 0.5 * float(np.log(D)))
    ones64 = pool.tile([128, 64], BF16, name="ones64")
    nc.sync.dma_start(ones64[:], d["d_ones64"].ap()[:, :])
    ident = pool.tile([128, 128], F32, name="ident")
    make_identity(nc, ident[:])
    ident16 = pool.tile([128, 128], BF16, name="ident16")
    make_identity(nc, ident16[:])
    maskc = pool.tile([128, NST], F32, name="maskc")
    nc.sync.dma_start(maskc[:], d["d_mask"].ap().rearrange("(n p) -> p n", p=128))

    # ---- persistent activations ------------------------------------------
    xT = pool.tile([128, NT, S], BF16, name="xT")       # layer input, feature-major
    aT = pool.tile([128, NT, S], BF16, name="aT")       # post-attn LN out
    qT = pool.tile([128, NT, S], BF16, name="qT")
    kT = pool.tile([128, NT, S], BF16, name="kT")
    cT = pool.tile([128, NT, S], BF16, name="cT")       # ctx, feature-major
    ybuf = pool.tile([128, NT, S], F32R, name="ybuf")   # pre-LN staging
    vpack = pool.tile([128, NST, H, DK], BF16, name="vpack")

    # =======================================================================
    # layernorm along the feature (partition-spread) dim, feature-major.
    # y: [128, nt, S] F32R tile; writes out[:, dt, :].
    # st0/st1/bc0/bc1: psum tiles supplied by the caller (bank-tag plan).
    # gamma applied via STT with the rstd broadcast; beta is folded into the
    # next layer's biases unless b_col is given (emb=None, last layer=fbc).
    def layernorm(y, nt, eps, out, dim, st0, st1, bc0, bc1,
                  g_col, b_col=None):
        sq_t = []
        for dt in range(nt):
            sqt = spool.tile([128, S], BF16, name=f"sq{dt}", tag=f"sq{dt % 2}")
            nc.scalar.activation(sqt[:], y[:, dt, :].bitcast(F32), AF.Square)
            sq_t.append(sqt)
        for dt in range(nt):
            nc.tensor.matmul(st0[:], ones_col[:], y[:, dt, :],
                             start=(dt == 0), stop=(dt == nt - 1))
        for dt in range(nt):
            nc.tensor.matmul(st1[:], ones_colb[:], sq_t[dt][:],
                             start=(dt == 0), stop=(dt == nt - 1))
        ln_chain(nt, eps, dim, y, out, st0, st1, bc0, bc1, g_col, b_col)

    HALF_LN_D = 0.5 * float(np.log(D))

    def ln_chain(nt, eps, dim, y, out, st0, st1, bc0, bc1, g_col, b_col):
        # var*dim = st1 - st0^2/dim; rstd = exp(-.5*ln(var*dim) + .5*ln(dim))
        mu = spool.tile([1, S], F32R, name="mu", tag="ln_mu")
        nc.vector.tensor_scalar(mu[:], st0[:], 1.0 / dim, None, OP.mult)
        ssq = spool.tile([1, S], F32R, name="ssq", tag="ln_ssq")
        nc.vector.tensor_tensor(ssq[:], mu[:].bitcast(F32), mu[:].bitcast(F32),
                                op=OP.mult)
        vard = spool.tile([1, S], F32R, name="vard", tag="ln_vard")
        nc.vector.scalar_tensor_tensor(vard[:], ssq[:].bitcast(F32), -float(dim),
                                       st1[:], op0=OP.mult, op1=OP.add)
        nc.tensor.matmul(bc0[:], neg_row[:], mu[:], start=True, stop=True)
        bc0c = spool.tile([128, S], F32, name="bc0c", tag="bc0c")
        nc.scalar.activation(bc0c[:], bc0[:], AF.Identity)
        lnv = spool.tile([1, S], F32R, name="lnv", tag="ln_lnv")
        nc.scalar.activation(lnv[:], vard[:].bitcast(F32), AF.Ln)
        rstd = spool.tile([1, S], F32R, name="rstd", tag="ln_rstd")
        nc.scalar.activation(rstd[:], lnv[:].bitcast(F32), AF.Exp, scale=-0.5,
                             bias=halfln[:, 0:1])
        nc.tensor.matmul(bc1[:], one_row[:], rstd[:], start=True, stop=True)
        for dt in range(nt):
            yc = spool.tile([128, S], F32, name=f"yc{dt}", tag=f"yc{dt % 2}")
            nc.gpsimd.tensor_tensor(yc[:], y[:, dt, :].bitcast(F32),
                                    bc0c[:], op=OP.add)
            nc.vector.scalar_tensor_tensor(
                out[:, dt, :], yc[:], g_col[:, dt:dt + 1],
                bc1[:], op0=OP.mult, op1=OP.mult)
            if b_col is not None:
                nc.vector.tensor_scalar(out[:, dt, :], out[:, dt, :],
                                        b_col[:, dt:dt + 1], None, OP.add)

    # =======================================================================
    # embedding: gather + add + transpose to feature-major + LN -> xT
    with tc.tile_pool(name="emb_sb", bufs=1) as embp:
        egc = ppool.tile([128, NT], F32, name="egc")
        nc.sync.dma_start(egc[:], d["d_eg"].ap().rearrange("(n p) -> p n", p=128))
        # issue ALL input DMAs/gathers up front; ids/tti/pos go on the idle
        # scalar queue so nothing serializes behind the sync queue's bursts
        dtb = ppool.tile([128, D], BF16, name="dtb")
        nc.scalar.dma_start(dtb[:], d["d_dtb"].ap()[:, :])
        x0s, tts, pgs = [], [], []
        for st in range(NST):
            idst = embp.tile([128, 1], I32, name="idst", tag=f"idst{st}")
            nc.scalar.dma_start(idst[:], d["d_ids"].ap()[st * 128:(st + 1) * 128, :])
            ttst = embp.tile([128, 1], F32, name="ttst", tag=f"ttst{st}")
            nc.scalar.dma_start(ttst[:], d["d_tti"].ap()[st * 128:(st + 1) * 128, :])
            x0 = embp.tile([128, D], F32, name="x0", tag=f"x0{st}")
            nc.gpsimd.indirect_dma_start(
                out=x0[:], out_offset=None, in_=d["d_wemb"].ap(),
                in_offset=bass.IndirectOffsetOnAxis(ap=idst[:, :1], axis=0))
            pg = embp.tile([128, D], F32, name="pg", tag=f"pg{st}")
            nc.scalar.dma_start(pg[:], d["d_pemb"].ap()[st * 128:(st + 1) * 128, :])
            x0s.append(x0)
            tts.append(ttst)
            pgs.append(pg)
        for st in range(NST):
            x0 = x0s[st]
            # type embedding has 2 rows: temb[0] folded into pos_emb, and
            # the delta applied as a rank-1 update tt[p] * (temb1-temb0)[f]
            nc.vector.scalar_tensor_tensor(x0[:], dtb[:], tts[st][:, 0:1],
                                           x0[:], op0=OP.mult, op1=OP.add)
            nc.vector.tensor_tensor(x0[:], x0[:], pgs[st][:], op=OP.add)
            for dt in range(NT):
                trp = ps_s.tile([128, 128], F32, name="trp", tag=f"s{dt % 2}")
                nc.tensor.transpose(trp[:], x0[:, dt * 128:(dt + 1) * 128], ident[:])
                nc.vector.tensor_copy(ybuf[:, dt, st * 128:(st + 1) * 128], trp[:])
        est0 = ps_a.tile([1, S], F32, name="est0", tag="a0")
        est1 = ps_a.tile([1, S], F32, name="est1", tag="a1")
        ebc0 = ps_s.tile([128, S], F32, name="ebc0", tag="s0")
        ebc1 = ps_s.tile([128, S], F32, name="ebc1", tag="s1")
        layernorm(ybuf, NT, 1e-12, xT, D, est0, est1, ebc0, ebc1,
                  g_col=egc[:])

    # =======================================================================
    # transformer layers
    for l in range(L):
        # ---- per-layer params -------------------------------------------
        bqc = ppool.tile([128, NT], F32, name="bqc", tag="bqc")
        nc.scalar.dma_start(bqc[:], d["d_bq"].ap()[l].rearrange("(n p) -> p n", p=128))
        bkc = ppool.tile([128, NT], F32, name="bkc", tag="bkc")
        nc.scalar.dma_start(bkc[:], d["d_bk"].ap()[l].rearrange("(n p) -> p n", p=128))
        bvb = ppool.tile([128, 3, 256], BF16, name="bvb", tag="bvb")
        nc.scalar.dma_start(bvb[:], d["d_bvb"].ap()[l].rearrange("p (c e) -> p c e", c=3))
        boc = ppool.tile([128, NT], F32, name="boc", tag="boc")
        nc.scalar.dma_start(boc[:], d["d_bo"].ap()[l].rearrange("(n p) -> p n", p=128))
        b2c = ppool.tile([128, NT], F32, name="b2c", tag="b2c")
        nc.scalar.dma_start(b2c[:], d["d_b2"].ap()[l].rearrange("(n p) -> p n", p=128))
        fgc = ppool.tile([128, NT], F32, name="fgc", tag="fgc")
        nc.scalar.dma_start(fgc[:], d["d_fg"].ap()[l].rearrange("(n p) -> p n", p=128))
        if l == L - 1:
            fbc = ppool.tile([128, NT], F32, name="fbc", tag="fbc")
            nc.scalar.dma_start(fbc[:], d["d_fb"].ap()[l].rearrange("(n p) -> p n", p=128))

        # ---- phase 1: interleaved QKV + attention ------------------------
        # e_tiles[p] = list of 8 exp tiles (kt-major, 2 heads per kt)
        e_tiles = [[None] * 8 for _ in range(NP)]
        dps_t = [None] * NP
        cps_t = [None] * NP
        rsb_t = [None] * NP
        wv_t = [None] * 3

        def emit_v_st(c, st):
            if st == 0:
                wv = wvpool.tile([128, NT, 256], BF16, name=f"wv{c}", tag="wv")
                nc.sync.dma_start(wv[:], d["d_wv"].ap()[l, c])
                wv_t[c] = wv
            acc = ps_a.tile([128, 256], F32, name=f"v{st}", tag=f"a{st % 2}")
            for dt in range(NT):
                nc.tensor.matmul(acc[:], xT[:, dt, st * 128:(st + 1) * 128],
                                 wv_t[c][:, dt, :],
                                 start=(dt == 0), stop=(dt == NT - 1))
            nc.vector.tensor_tensor(
                vpack[:, st, c * 4:(c + 1) * 4, :],
                acc[:].rearrange("p (a b) -> p a b", a=4),
                bvb[:, c, :].rearrange("p (a b) -> p a b", a=4),
                op=OP.add)

        def emit_qk1(et, which):
            wd, bcol, dst, nm = ((d["d_wq"], bqc, qT, "q") if which == 0
                                 else (d["d_wk"], bkc, kT, "k"))
            wt = wqkpool.tile([128, NT, 128], BF16, name=f"w{nm}{et}", tag="wqk")
            nc.sync.dma_start(wt[:], wd.ap()[l, et])
            acc = ps_a.tile([128, S], F32, name=f"{nm}{et}", tag=f"a{et % 2}")
            for dt in range(NT):
                nc.tensor.matmul(acc[:], wt[:, dt, :], xT[:, dt, :],
                                 start=(dt == 0), stop=(dt == NT - 1))
            nc.scalar.activation(dst[:, et, :], acc[:], AF.Identity,
                                 bias=bcol[:, et:et + 1])

        def emit_sc(p, kt):
            scs = []
            for hh in range(2):
                lo = hh * 64
                sc = ps_s.tile([128, S], F32, name=f"sc{kt}{hh}", tag=f"s{hh}")
                nc.tensor.matmul(
                    sc[:], kT[lo:lo + 64, p, kt * 128:(kt + 1) * 128],
                    qT[lo:lo + 64, p, :], start=True, stop=True,
                    tile_position=(lo, 0))
                scs.append(sc)
            for hh in range(2):
                et = epool.tile([128, S], BF16, name=f"e{kt}{hh}", tag="e")
                nc.scalar.activation(et[:], scs[hh][:], AF.Exp,
                                     bias=maskc[:, kt:kt + 1])
                e_tiles[p][kt * 2 + hh] = et

        def emit_dpv(p, kt):
            # denominator (64-row-replicated colsum) + PV, both col-group
            # packed: head A -> partitions 0-63, head B -> 64-127.
            if kt == 0:
                dps_t[p] = ps_d.tile([128, S], F32, name=f"dps{p}", tag=f"d{p % 2}")
                cps_t[p] = ps_c.tile([128, S], F32, name=f"cps{p}", tag=f"c{p % 2}")
            dps, cps = dps_t[p], cps_t[p]
            for hh in range(2):
                e = e_tiles[p][kt * 2 + hh]
                nc.tensor.matmul(dps[64 * hh:64 * hh + 64, :], ones64[:], e[:],
                                 start=(kt == 0), stop=(kt == NST - 1),
                                 tile_position=(0, 64 * hh),
                                 skip_group_check=True)
            for hh in range(2):
                e = e_tiles[p][kt * 2 + hh]
                nc.tensor.matmul(cps_t[p][64 * hh:64 * hh + 64, :],
                                 vpack[:, kt, 2 * p + hh, :], e[:],
                                 start=(kt == 0), stop=(kt == NST - 1),
                                 tile_position=(0, 64 * hh),
                                 skip_group_check=True)

        def emit_recip(p):
            # rsb = 1/denom = exp(-ln(d)) on ACT (rows already per-head)
            nl = spool.tile([128, S], F32, name=f"nl{p}", tag=f"nl{p % 2}")
            nc.scalar.activation(nl[:], dps_t[p][:], AF.Ln)
            rsb = spool.tile([128, S], F32, name=f"rsb{p}", tag=f"rsb{p % 2}")
            nc.scalar.activation(rsb[:], nl[:], AF.Exp, scale=-1.0)
            rsb_t[p] = rsb

        def emit_ct(p):
            nc.vector.tensor_tensor(cT[:, p, :], cps_t[p][:], rsb_t[p][:],
                                    op=OP.mult)
            e_tiles[p] = None

        # Wo accumulation waves: wacc[et] accumulates dt chunks as cT tiles
        # land. Bank tags chosen to be free at each wave's position.
        WO_TAGS = ["a0", "a1", "s0", "s1", "c0", "d0"]
        wacc = [None] * NT
        wo_w = [None] * NT

        for _et in range(NT):
            _wt = wopool.tile([128, NT, 128], BF16, name=f"wo{_et}", tag="wo")
            nc.sync.dma_start(_wt[:], d["d_wo"].ap()[l, _et])
            wo_w[_et] = _wt

        def emit_wo(et, dts):
            if wacc[et] is None:
                wacc[et] = (ps_a if WO_TAGS[et].startswith("a") else
                            ps_s if WO_TAGS[et].startswith("s") else
                            ps_c if WO_TAGS[et].startswith("c") else ps_d
                            ).tile([128, S], F32, name=f"o{et}", tag=WO_TAGS[et])
            for dt in dts:
                nc.tensor.matmul(wacc[et][:], wo_w[et][:, dt, :], cT[:, dt, :],
                                 start=(dt == 0), stop=(dt == NT - 1),
                                 skip_group_check=True)

        # interleaved emission: fine-grained so the PE never sits behind a
        # score matmul waiting for the ACT exp of the previous kt.
        emit_v_st(0, 0)
        emit_v_st(0, 1)
        emit_qk1(0, 0)
        emit_qk1(0, 1)
        emit_v_st(0, 2)
        emit_v_st(0, 3)
        emit_qk1(1, 0)
        emit_qk1(1, 1)
        # pair 0
        emit_sc(0, 0)
        emit_v_st(1, 0)
        emit_sc(0, 1)
        emit_dpv(0, 0)
        emit_v_st(1, 1)
        emit_sc(0, 2)
        emit_dpv(0, 1)
        emit_v_st(1, 2)
        emit_sc(0, 3)
        emit_dpv(0, 2)
        emit_v_st(1, 3)
        emit_dpv(0, 3)
        emit_recip(0)
        emit_qk1(2, 0)
        emit_ct(0)
        # pair 1
        emit_sc(1, 0)
        emit_qk1(2, 1)
        emit_sc(1, 1)
        emit_dpv(1, 0)
        emit_qk1(3, 0)
        emit_sc(1, 2)
        emit_dpv(1, 1)
        emit_qk1(3, 1)
        emit_sc(1, 3)
        emit_dpv(1, 2)
        emit_v_st(2, 0)
        emit_dpv(1, 3)
        emit_recip(1)
        emit_v_st(2, 1)
        emit_ct(1)
        # pair 2
        emit_sc(2, 0)
        emit_v_st(2, 2)
        emit_sc(2, 1)
        emit_dpv(2, 0)
        emit_v_st(2, 3)
        emit_sc(2, 2)
        emit_dpv(2, 1)
        emit_qk1(4, 0)
        emit_sc(2, 3)
        emit_dpv(2, 2)
        emit_qk1(4, 1)
        emit_dpv(2, 3)
        emit_recip(2)
        emit_qk1(5, 0)
        emit_ct(2)
        # pair 3 (wacc et0/et1 live on a0/a1 from here: QKV accs are done)
        emit_sc(3, 0)
        emit_qk1(5, 1)
        emit_sc(3, 1)
        emit_dpv(3, 0)
        emit_wo(0, [0, 1, 2])
        emit_sc(3, 2)
        emit_dpv(3, 1)
        emit_wo(1, [0, 1, 2])
        emit_sc(3, 3)
        emit_dpv(3, 2)
        emit_dpv(3, 3)
        emit_recip(3)
        emit_ct(3)
        # pair 4
        emit_sc(4, 0)
        emit_wo(0, [3])
        emit_wo(1, [3])
        emit_sc(4, 1)
        emit_dpv(4, 0)
        emit_sc(4, 2)
        emit_dpv(4, 1)
        emit_sc(4, 3)
        emit_dpv(4, 2)
        emit_dpv(4, 3)
        emit_recip(4)
        emit_ct(4)
        # pair 5 (et4 tag c0 free once ct(4) drained; et5 tag d0 free
        # once recip(4) has read dps4)
        emit_sc(5, 0)
        emit_wo(4, [0, 1, 2, 3])
        emit_sc(5, 1)
        emit_dpv(5, 0)
        emit_wo(5, [0, 1, 2])
        emit_sc(5, 2)
        emit_dpv(5, 1)
        emit_wo(0, [4])
        emit_wo(1, [4])
        emit_sc(5, 3)
        emit_wo(5, [3, 4])
        emit_dpv(5, 2)
        emit_wo(4, [4])
        emit_dpv(5, 3)
        emit_recip(5)
        emit_ct(5)
        # s0/s1 banks are free after the last exp of pair 5
        emit_wo(2, [0, 1, 2, 3, 4])
        emit_wo(3, [0, 1, 2, 3, 4])
        emit_wo(0, [5])
        emit_wo(1, [5])
        emit_wo(2, [5])
        emit_wo(3, [5])
        emit_wo(4, [5])
        emit_wo(5, [5])

        # ---- Wo epilogue + LN1 (gamma/beta folded into W1/b2) ------------
        lst0 = ps_d.tile([1, S], F32, name="lst0", tag="d1")
        lst1 = ps_c.tile([1, S], F32, name="lst1", tag="c1")
        for et in range(NT):
            nc.vector.scalar_tensor_tensor(
                ybuf[:, et, :], wacc[et][:], boc[:, et:et + 1],
                xT[:, et, :], op0=OP.add, op1=OP.add)
            sqt = spool.tile([128, S], BF16, name=f"sq{et}", tag=f"sq{et % 2}")
            nc.scalar.activation(sqt[:], ybuf[:, et, :].bitcast(F32), AF.Square)
            nc.tensor.matmul(lst0[:], ones_col[:], ybuf[:, et, :],
                             start=(et == 0), stop=(et == NT - 1),
                             skip_group_check=True)
            nc.tensor.matmul(lst1[:], ones_colb[:], sqt[:],
                             start=(et == 0), stop=(et == NT - 1),
                             skip_group_check=True)
        lbc0 = ps_a.tile([128, S], F32, name="lbc0", tag="a0")
        lbc1 = ps_a.tile([128, S], F32, name="lbc1", tag="a1")
        # LN1: gamma/beta folded into W1/b2 -> plain 2-op apply
        mu = spool.tile([1, S], F32R, name="mu", tag="ln_mu")
        nc.vector.tensor_scalar(mu[:], lst0[:], 1.0 / D, None, OP.mult)
        ssq = spool.tile([1, S], F32R, name="ssq", tag="ln_ssq")
        nc.vector.tensor_tensor(ssq[:], mu[:].bitcast(F32), mu[:].bitcast(F32),
                                op=OP.mult)
        vard = spool.tile([1, S], F32R, name="vard", tag="ln_vard")
        nc.vector.scalar_tensor_tensor(vard[:], ssq[:].bitcast(F32), -float(D),
                                       lst1[:], op0=OP.mult, op1=OP.add)
        nc.tensor.matmul(lbc0[:], neg_row[:], mu[:], start=True, stop=True)
        lbc0c = spool.tile([128, S], F32, name="lbc0c", tag="bc0c")
        nc.scalar.activation(lbc0c[:], lbc0[:], AF.Identity)
        lnv = spool.tile([1, S], F32R, name="lnv", tag="ln_lnv")
        nc.scalar.activation(lnv[:], vard[:].bitcast(F32), AF.Ln)
        rstd = spool.tile([1, S], F32R, name="rstd", tag="ln_rstd")
        nc.scalar.activation(rstd[:], lnv[:].bitcast(F32), AF.Exp, scale=-0.5,
                             bias=halfln[:, 0:1])
        nc.tensor.matmul(lbc1[:], one_row[:], rstd[:], start=True, stop=True)
        for dt in range(NT):
            yc = spool.tile([128, S], F32, name=f"lyc{dt}", tag=f"yc{dt % 2}")
            nc.gpsimd.tensor_tensor(yc[:], ybuf[:, dt, :].bitcast(F32),
                                    lbc0c[:], op=OP.add)
            nc.vector.tensor_tensor(aT[:, dt, :], yc[:],
                                    lbc1[:], op=OP.mult)

        # ---- phase 4: FFN (W1 -> h, W2 accumulate into 6 yT banks) -------
        YT = ["s0", "s1", "c0", "c1", "d0", "d1"]
        ytiles = []
        for et in range(NT):
            yt = (ps_s if YT[et].startswith("s") else
                  ps_c if YT[et].startswith("c") else ps_d
                  ).tile([128, S], F32, name=f"yt{et}", tag=YT[et])
            ytiles.append(yt)
        h_sb = [None] * NTF
        w2ts = [None] * NTF

        def emit_h(f):
            c, fj = divmod(f, 2)
            if fj == 0:
                w1t = w1pool.tile([128, NT, 256], BF16, name=f"w1_{c}", tag="w1")
                nc.sync.dma_start(w1t[:], d["d_w1"].ap()[l, c])
                emit_h.w1t = w1t
                w2t = w2pool.tile([128, 2, D], BF16, name=f"w2_{c}", tag="w2")
                nc.sync.dma_start(w2t[:], d["d_w2"].ap()[l, c])
                emit_h.w2t = w2t
            hacc = ps_a.tile([128, S], F32, name=f"h{f}", tag=f"a{f % 2}")
            for dt in range(NT):
                nc.tensor.matmul(hacc[:], emit_h.w1t[:, dt, fj * 128:(fj + 1) * 128],
                                 aT[:, dt, :], start=(dt == 0), stop=(dt == NT - 1))
            hs = hpool.tile([128, S], BF16, name=f"hs{f}", tag="hs")
            nc.vector.tensor_copy(hs[:], hacc[:])
            h_sb[f] = hs
            w2ts[f] = emit_h.w2t

        def emit_y(f):
            fj = f % 2
            w2t = w2ts[f]
            for et in range(NT):
                nc.tensor.matmul(ytiles[et][:],
                                 w2t[:, fj, et * 128:(et + 1) * 128],
                                 h_sb[f][:], start=(f == 0), stop=(f == NTF - 1),
                                 skip_group_check=True)
            h_sb[f] = None

        emit_h(0)
        for f in range(1, NTF):
            emit_h(f)
            emit_y(f - 1)
        emit_y(NTF - 1)

        # ---- LN2 entry: per-et +b2 / square / stats, DVE-ACT alternated
        nst0 = ps_a.tile([1, S], F32, name="nst0", tag="a0")
        nst1 = ps_a.tile([1, S], F32, name="nst1", tag="a1")
        for et in range(NT):
            nc.vector.tensor_scalar(ybuf[:, et, :], ytiles[et][:],
                                    b2c[:, et:et + 1], None, OP.add)
            # square computed straight off the PSUM ytile, +b2 via ACT bias:
            # runs in parallel with the DVE drains instead of behind them
            sqt = spool.tile([128, S], BF16, name=f"nsq{et}", tag=f"sq{et % 2}")
            nc.scalar.activation(sqt[:], ytiles[et][:], AF.Square,
                                 bias=b2c[:, et:et + 1])
            nc.tensor.matmul(nst0[:], ones_col[:], ybuf[:, et, :],
                             start=(et == 0), stop=(et == NT - 1),
                             skip_group_check=True)
            nc.tensor.matmul(nst1[:], ones_colb[:], sqt[:],
                             start=(et == 0), stop=(et == NT - 1),
                             skip_group_check=True)

        # ---- LN2 chain -> xT (next layer input); beta folded fwd except l=11
        nbc0 = ps_s.tile([128, S], F32, name="nbc0", tag="s0")
        nbc1 = ps_s.tile([128, S], F32, name="nbc1", tag="s1")
        ln_chain(NT, 1e-5, D, ybuf, xT, nst0, nst1, nbc0, nbc1,
                 g_col=fgc[:], b_col=(fbc[:] if l == L - 1 else None))

    # =======================================================================
    # output: transpose xT -> [S, D] and DMA out
    with tc.tile_pool(name="out_sb", bufs=2) as outp:
        for st in range(NST):
            ops_t = ps_s.tile([128, D], BF16, name="ops", tag=f"s{st % 2}")
            for dt in range(NT):
                nc.tensor.transpose(ops_t[:, dt * 128:(dt + 1) * 128],
                                    xT[:, dt, st * 128:(st + 1) * 128],
                                    ident16[:])
            osb = outp.tile([128, D], BF16, name="osb", tag="osb")
            nc.vector.tensor_copy(osb[:], ops_t[:])
            nc.sync.dma_start(d["d_out"].ap()[st * 128:(st + 1) * 128, :], osb[:])


# ---------------------------------------------------------------------------
def kernel(**inputs):
    global _NC_CACHE
    if _NC_CACHE is None:
        _NC_CACHE = _build_nc()
    nc = _NC_CACHE

    import ml_dtypes
    f32 = lambda a: np.ascontiguousarray(np.asarray(a), dtype=np.float32)
    bf = lambda a: np.ascontiguousarray(a.astype(ml_dtypes.bfloat16))

    Wq = f32(inputs["Wq"])
    bq = f32(inputs["bq"])
    Wk, Wv, Wo = f32(inputs["Wk"]), f32(inputs["Wv"]), f32(inputs["Wo"])
    W1, W2 = f32(inputs["W1"]), f32(inputs["W2"])
    ag, ab = f32(inputs["attn_ln_g"]), f32(inputs["attn_ln_b"])
    fg, fb = f32(inputs["ffn_ln_g"]), f32(inputs["ffn_ln_b"])
    eg, eb = f32(inputs["emb_ln_g"]), f32(inputs["emb_ln_b"])
    b1, b2 = f32(inputs["b1"]), f32(inputs["b2"])
    bk, bv, bo = f32(inputs["bk"]), f32(inputs["bv"]), f32(inputs["bo"])

    # fold LN1 gamma into W1 columns, beta into b1; then b1 into b2
    W1f = W1 * ag[:, None, :]                    # [L,F,D] * [L,1,D]
    b1f = b1 + np.einsum("lfd,ld->lf", W1, ab)
    b2f = b2 + np.einsum("ldf,lf->ld", W2, b1f)

    # fold the PREVIOUS LN2 (or emb-LN) beta into this layer's QKV biases
    # and the residual bias: x_true = x_core + prev_beta (per-feature).
    prev_beta = np.concatenate([eb[None, :], fb[:-1]], axis=0)   # [L, D]
    bq_f = bq + np.einsum("led,ld->le", Wq, prev_beta)
    bk_f = bk + np.einsum("led,ld->le", Wk, prev_beta)
    bv_f = bv + np.einsum("led,ld->le", Wv, prev_beta)
    bo_f = bo + prev_beta

    # fold the attention scale into Q
    Wq_s = Wq * SCALE
    bq_s = bq_f * SCALE

    def tile_qk(W):  # [L, Dout, Din] -> [L, et, p, n, e]
        WT = W.transpose(0, 2, 1)                # [L, Din, Dout]
        return bf(WT.reshape(L, NT, 128, NT, 128).transpose(0, 3, 2, 1, 4))

    def tile_v(W):   # -> [L, c3, p, n, e256]
        WT = W.transpose(0, 2, 1)
        return bf(WT.reshape(L, NT, 128, 3, 256).transpose(0, 3, 2, 1, 4))

    def tile_w1(W):  # [L, F, D] -> [L, c12, p, n6, e256]
        WT = W.transpose(0, 2, 1)                # [L, D, F]
        return bf(WT.reshape(L, NT, 128, NTF // 2, 256).transpose(0, 3, 2, 1, 4))

    def tile_w2(W):  # [L, D, F] -> [L, c12, p, g2, e768]
        WT = W.transpose(0, 2, 1)                # [L, F, D]
        return bf(WT.reshape(L, NTF // 2, 2, 128, D).transpose(0, 1, 3, 2, 4))

    bvb = np.broadcast_to(bv_f[:, None, :], (L, 128, D))  # [L, 128, D]

    temb = f32(inputs["type_emb"])
    shared = {
        "wemb": f32(inputs["word_emb"]),
        "pemb": f32(inputs["pos_emb"])[:S] + temb[0][None, :],
        "dtb": bf(np.broadcast_to(temb[1] - temb[0], (128, D))),
        "eg": eg,
        "wq": tile_qk(Wq_s),
        "wk": tile_qk(Wk),
        "wv": tile_v(Wv),
        "wo": tile_qk(Wo),
        "w1": tile_w1(W1f),
        "w2": tile_w2(W2),
        "bq": bq_s, "bk": bk_f,
        "bvb": bf(bvb),
        "bo": bo_f, "b2": b2f,
        "fg": fg, "fb": fb,
        "ones128": np.ones(128, np.float32),
        "ones128b": np.ones(128, ml_dtypes.bfloat16),
        "ones64": np.ones((128, 64), ml_dtypes.bfloat16),
        "neg1": np.full((1, 128), -1.0, np.float32),
    }
    ids = np.asarray(inputs["input_ids"]).astype(np.int32)
    tti = np.asarray(inputs["token_type_ids"]).astype(np.float32)
    am = np.asarray(inputs["attention_mask"]).astype(np.float32)
    in_maps = []
    for c in range(B):
        in_maps.append({
            **shared,
            "ids": ids[c].reshape(S, 1),
            "tti": tti[c].reshape(S, 1),
            "maskadd": np.where(am[c] == 0, -1e9, 0.0).astype(np.float32),
        })
    res = bass_utils.run_bass_kernel_spmd(
        nc, in_maps, core_ids=list(range(B)), trace=False)
    out = np.stack([res.results[c]["out"] for c in range(B)], axis=0)
    return out.astype(np.float32)
